# revision 11
# baseline (speedup 1.0000x reference)
"""Trainium2 Bass kernel for MinibatchDiscrimination.

Computes out = concat([x, F], axis=1) where
  act = einsum('bd,kdm->bkm', x, W)               (B, K, M)
  D[b,k,c] = sum_m |act[b,k,m] - act[c,k,m]|      (B, K, B)
  F[b,k] = sum_c exp(-D[b,k,c])                   (B, K)

Sharding: tensor-parallel over nb_kernels (K=32) -> 4 kernels per core on 8
cores. Each core computes its F slice (B, 4); host concatenates [x | F].

Per-core algorithm:
  * actT[(k,m), b] = W_slice^T x^T via PE (bf16 in, f32 accum).
  * Pairwise L1 split between engines: planes on the DVE use
    |x| = 2 relu(x) - x (relu tensor_scalar at 4x) with the -x part
    handled in factored form (rank-1 rs correction + exp bias); planes
    on ACT use Abs directly (|x|/2, needs no correction).  The per-k
    DVE/ACT plane split is asymmetric (k 0,1: ACT gets plane 7; k 2,3:
    planes 6,7) to balance engine load, and blockdiag/rs cover only the
    DVE planes' m values per k.
  * m-contraction on the PE with a one-hot lhsT producing 64 outputs;
    the G=0/G=1 half-slabs go to PSUM column-halves (tile_position
    (0,0)/(0,64)) and are interleaved in issue order so the two column
    groups of the array stream concurrently (~2x PE throughput).
  * The rank-1 rs_c correction opens each slab's accumulation group as a
    single K=4, M=128 matmul.
  * E = exp(-2P + bias) on ACT with accum_out giving row sums.  The
    mirrored column sums accumulate across slabs into one (8, B) PSUM
    tile (lhsT = one-hot column per slab; slab 0 opens rows as zeros),
    copied out 8-wide once per k, bounced via DRAM to transpose into
    per-partition layout, reduced on the DVE.
"""

import sys

if "/opt/trn_rl_repo" not in sys.path:
    sys.path.insert(0, "/opt/trn_rl_repo")

import numpy as np
import ml_dtypes

NB_KERNELS = 32
KERNEL_DIM = 16  # M
INPUT_DIM = 1024  # D
BATCH = 1024  # B
N_CORES = 8
KPC = NB_KERNELS // N_CORES  # kernels per core = 4

# planes handled by ACT (as |x|/2 via Abs) per k; the rest go to the DVE
# (as relu(x)).  blockdiag / the rs correction cover only DVE planes' m's.
ACT_PLANES = {0: (7,), 1: (7,), 2: (6, 7), 3: (6, 7)}

_CACHE = {}


def _build_program():
    import concourse.bass as bass
    import concourse.tile as tile
    from concourse import bacc, mybir

    bf16 = mybir.dt.bfloat16
    f32 = mybir.dt.float32
    Alu = mybir.AluOpType
    Act = mybir.ActivationFunctionType

    nc = bacc.Bacc(
        "TRN2",
        target_bir_lowering=False,
        debug=False,
        enable_asserts=False,
        num_devices=N_CORES,
    )

    KM = KPC * KERNEL_DIM  # 64 rows of actT
    NSLAB = BATCH // 128  # 8 slabs of b

    xT = nc.dram_tensor("xT", (INPUT_DIM, BATCH), bf16, kind="ExternalInput").ap()
    wT = nc.dram_tensor("wT", (INPUT_DIM, KM), bf16, kind="ExternalInput").ap()
    onehot = nc.dram_tensor("onehot", (128, 64), bf16, kind="ExternalInput").ap()
    # blockdiag[(k,m), k'] = 1 if k == k' and m is a DVE plane of k
    blockdiag = nc.dram_tensor("blockdiag", (KM, KPC), bf16, kind="ExternalInput").ap()
    # selk128[q, k*128 + p] = 1 if q == k (rank-1 rs correction, M=128)
    selk128 = nc.dram_tensor("selk128", (KPC, KPC * 128), bf16, kind="ExternalInput").ap()
    f_out = nc.dram_tensor("f_out", (BATCH, KPC), f32, kind="ExternalOutput").ap()

    DCH = INPUT_DIM // 128  # 8 chunks of the matmul contraction dim

    def mm_chunks(span):
        off = 0
        while off < span:
            fd = min(512, span - off)
            yield off, fd
            off += fd

    with tile.TileContext(nc) as tc:
        with (
            tc.tile_pool(name="singles", bufs=1) as singles,
            tc.tile_pool(name="vk", bufs=3) as vk_pool,
            tc.tile_pool(name="sk", bufs=3) as sk_pool,
            tc.tile_pool(name="tg", bufs=8) as t_pool,
            tc.tile_pool(name="es", bufs=3) as e_pool,
            tc.tile_pool(name="fk", bufs=2) as fk_pool,
            tc.tile_pool(name="small", bufs=4) as small_pool,
            tc.tile_pool(name="dps", bufs=2, space="PSUM") as d_psum,
            tc.tile_pool(name="csps", bufs=2, space="PSUM") as cs_psum,
            tc.tile_pool(name="dram", bufs=1, space="DRAM") as dram_pool,
            tc.tile_pool(name="dramk", bufs=2, space="DRAM") as dramk_pool,
        ):
            # ---- Phase 1: actT = wT^T @ xT on PE; rs prep ----
            xT_sb = singles.tile([128, DCH, BATCH], bf16)
            xT_r = xT.rearrange("(i p) b -> p i b", p=128)
            for i in range(DCH):
                eng = nc.scalar if i % 2 else nc.sync
                eng.dma_start(out=xT_sb[:, i, :], in_=xT_r[:, i, :])
            wT_sb = singles.tile([128, DCH, KM], bf16)
            nc.sync.dma_start(out=wT_sb[:], in_=wT.rearrange("(i p) c -> p i c", p=128))
            onehot_sb = singles.tile([128, 64], bf16)
            nc.sync.dma_start(out=onehot_sb[:], in_=onehot)
            blockdiag_sb = singles.tile([KM, KPC], bf16)
            nc.sync.dma_start(out=blockdiag_sb[:], in_=blockdiag)
            selk128_sb = singles.tile([KPC, KPC * 128], bf16)
            nc.sync.dma_start(out=selk128_sb[:], in_=selk128)
            ones_sb = singles.tile([128, 1], bf16)
            nc.vector.memset(ones_sb[:], 1.0)

            actT_sb = singles.tile([KM, BATCH], bf16)
            rsh_bf = singles.tile([KPC, BATCH], bf16)  # bf16(-rs/2)
            rs2_bf = singles.tile([KPC, BATCH], bf16)  # bf16(-rs) == 2*rsh_bf
            act_ps = d_psum.tile([KM, BATCH], f32, tag="D")
            for h in range(BATCH // 512):
                for i in range(DCH):
                    nc.tensor.matmul(
                        act_ps[:, h * 512 : (h + 1) * 512],
                        lhsT=wT_sb[:, i, :],
                        rhs=xT_sb[:, i, h * 512 : (h + 1) * 512],
                        start=(i == 0),
                        stop=(i == DCH - 1),
                    )
            nc.scalar.copy(actT_sb[:], act_ps[:])
            # rs[k, c] = sum over k's DVE-plane m's of bf16 actT[c, k, m]
            rs_ps = d_psum.tile([KPC, BATCH], f32, tag="D")
            for h in range(BATCH // 512):
                nc.tensor.matmul(
                    rs_ps[:, h * 512 : (h + 1) * 512],
                    lhsT=blockdiag_sb[:],
                    rhs=actT_sb[:, h * 512 : (h + 1) * 512],
                    start=True,
                    stop=True,
                )
            # bf16 rounding commutes with *2, so 2*rsh_bf == rs2_bf exactly;
            # the exp argument on the diagonal cancels to 0.
            nc.scalar.mul(rsh_bf[:], rs_ps[:], -0.5)
            nc.scalar.mul(rs2_bf[:], rs_ps[:], -1.0)

            # f32 upconvert of the bf16-rounded actT (tensor_scalar scalars
            # must be f32 but must equal V's bf16 values exactly).
            actT_f32 = singles.tile([KM, BATCH], f32)
            nc.vector.tensor_copy(actT_f32[:], actT_sb[:])

            # DRAM bounces: the broadcast/gather DMAs below need arbitrary
            # strided (incl. 0-step) source APs, which SBUF sources disallow.
            actT_dram = dram_pool.tile([KM, BATCH], bf16)
            nc.sync.dma_start(out=actT_dram[:], in_=actT_sb[:])
            actT32_dram = dram_pool.tile([KM, BATCH], f32)
            nc.sync.dma_start(out=actT32_dram[:], in_=actT_f32[:])
            rs2_dram = dram_pool.tile([KPC, BATCH], bf16)
            nc.sync.dma_start(out=rs2_dram[:], in_=rs2_bf[:])

            # bias_cols[p, k*NSLAB + slab] = bf16(-rs[k, slab*128 + p])
            bias_cols = singles.tile([128, KPC * NSLAB], bf16)
            rc0 = rs2_dram[0:1, 0:1]
            nc.sync.dma_start(
                out=bias_cols[:],
                in_=bass.AP(
                    tensor=rc0.tensor,
                    offset=rc0.offset,
                    ap=[[1, 128], [BATCH, KPC], [128, NSLAB]],
                ),
            )

            # ---- Phases 2+3 per kernel k ----
            # Partition layout of T tiles, per plane j in 0..7:
            #   partition p holds m = 2j + p//64 and b_loc = p % 64.
            # V[p, j, c] = actT_bf16[k*16 + 2j + p//64, c]
            # S[p, j, f] = actT_f32 [k*16 + 2j + p//64,
            #                        slab*128 + G*64 + p%64],  f = slab*2+G
            for k in range(KPC):
                base = k * KERNEL_DIM
                act_js = ACT_PLANES[k]
                # ACT planes first so the slower producer doesn't gate the
                # accumulation-group closers.
                j_order = list(act_js) + [j for j in range(8) if j not in act_js]
                Vp = {}
                S = sk_pool.tile([128, 8, 16], f32)
                for j in j_order:
                    Vp[j] = vk_pool.tile(
                        [128, BATCH], bf16, name=f"V{k}_{j}", tag=f"v{j}"
                    )
                    # one 3-dim broadcast DMA per plane: partition p gets
                    # actT row base+2j+(p//64); triggers alternate between
                    # the sync and gpsimd queues (descriptor generation for
                    # the 64-way replication is ~0.8us per trigger).
                    row_b = actT_dram[base + 2 * j : base + 2 * j + 1, 0:1]
                    veng = nc.sync if j % 2 else nc.gpsimd
                    veng.dma_start(
                        out=Vp[j][:],
                        in_=bass.AP(
                            tensor=row_b.tensor,
                            offset=row_b.offset,
                            ap=[[BATCH, 2], [0, 64], [1, BATCH]],
                        ),
                    )
                    for q in range(2):
                        row = base + 2 * j + q
                        row_s = actT32_dram[row : row + 1, 0:1]
                        nc.scalar.dma_start(
                            out=S[64 * q : 64 * (q + 1), j, :],
                            in_=bass.AP(
                                tensor=row_s.tensor,
                                offset=row_s.offset,
                                ap=[[1, 64], [128, NSLAB], [64, 2]],
                            ),
                        )
                # S_nh = -S/2: ACT-plane bias (Abs(0.5*V - 0.5*s) = |x|/2)
                S_nh = sk_pool.tile([128, 8, 16], f32, tag="snh")
                nc.vector.tensor_scalar(
                    out=S_nh.rearrange("p a b -> p (a b)"),
                    in0=S.rearrange("p a b -> p (a b)"),
                    scalar1=-0.5,
                    scalar2=None,
                    op0=Alu.mult,
                )

                # column-sum accumulator: cs_acc[0, c] accumulates, across
                # slabs s with 128*(s+1) <= c, the mirror mass
                # sum_b E_s[b, c].  One accumulation group spans the whole k
                # loop (slab 0 opens all of [128, B), slab 6 closes).
                cs_acc = cs_psum.tile([1, BATCH], f32)
                Fk = fk_pool.tile([128, NSLAB], f32)

                for slab in range(NSLAB):
                    c0 = slab * 128
                    span = BATCH - c0
                    # PSUM accumulates P = R + bf16(-rs[c]/2) (DVE planes)
                    # + sum |x|/2 (ACT planes).  Then exp(-2P + bias[b]) with
                    # bias[b] = bf16(-rs[b]) gives exp(-sum_m |act_b-act_c|),
                    # exactly 1 on the diagonal.
                    D = d_psum.tile([128, BATCH], f32)
                    # rank-1 rs correction opens the group across all 128
                    # partitions in one K=4, M=128 matmul.
                    for off, fd in mm_chunks(span):
                        nc.tensor.matmul(
                            D[:, off : off + fd],
                            lhsT=selk128_sb[:, k * 128 : (k + 1) * 128],
                            rhs=rsh_bf[:, c0 + off : c0 + off + fd],
                            start=True,
                            stop=False,
                        )
                    for j in j_order:
                        Tg = {}
                        for G in range(2):
                            scol = slab * 2 + G
                            T = t_pool.tile([128, BATCH], bf16, tag=f"t{G}")
                            if j in act_js:
                                nc.scalar.activation(
                                    out=T[:, :span],
                                    in_=Vp[j][:, c0:BATCH],
                                    func=Act.Abs,
                                    scale=0.5,
                                    bias=S_nh[:, j, scol : scol + 1],
                                )
                            else:
                                nc.vector.tensor_scalar(
                                    out=T[:, :span],
                                    in0=Vp[j][:, c0:BATCH],
                                    scalar1=S[:, j, scol : scol + 1],
                                    scalar2=0.0,
                                    op0=Alu.subtract,
                                    op1=Alu.max,
                                )
                            Tg[G] = T
                        last = j == j_order[-1]
                        # adjacent matmuls alternate PSUM column-halves
                        # (tile_position (0,0)/(0,64)) -> both column groups
                        # of the PE array stream concurrently.
                        for off, fd in mm_chunks(span):
                            for G in range(2):
                                nc.tensor.matmul(
                                    D[G * 64 : (G + 1) * 64, off : off + fd],
                                    lhsT=onehot_sb[:],
                                    rhs=Tg[G][:, off : off + fd],
                                    start=False,
                                    stop=last,
                                )
                    # E = exp(-D) over the slab's c-window; accum_out gives
                    # the row part sum_c E directly.
                    E = e_pool.tile([128, BATCH], bf16)
                    nc.scalar.activation(
                        out=E[:, :span],
                        in_=D[:, :span],
                        func=Act.Exp,
                        scale=-2.0,
                        bias=bias_cols[:, k * NSLAB + slab : k * NSLAB + slab + 1],
                        accum_out=Fk[:, slab : slab + 1],
                    )
                    # mirrored column parts for c in [c0+128, B): accumulate
                    # into the cross-slab cs_acc row.
                    if span > 128:
                        start = c0 + 128
                        while start < BATCH:
                            # keep each matmul within one PSUM bank (512 f32)
                            fd = min(512 - (start % 512), BATCH - start)
                            nc.tensor.matmul(
                                cs_acc[:, start : start + fd],
                                lhsT=ones_sb[:],
                                rhs=E[:, start - c0 : start - c0 + fd],
                                start=(slab == 0),
                                stop=(slab == NSLAB - 2)
                                and (start + fd >= BATCH),
                            )
                            start += fd

                # bounce the accumulated mirror row via DRAM into the
                # per-partition (p, t) layout and add into Fk
                colrow = small_pool.tile([1, BATCH - 128], f32, tag="colrow")
                nc.scalar.copy(colrow[:], cs_acc[0:1, 128:BATCH])
                rowdram = dramk_pool.tile([1, BATCH - 128], f32, tag="stg")
                nc.scalar.dma_start(out=rowdram[:], in_=colrow[:])
                # cadd[p, t] = rowdram[128*t + p] (t = 1..7 -> idx 0..6)
                cadd = small_pool.tile([128, NSLAB - 1], f32)
                st0 = rowdram[0:1, 0:1]
                nc.scalar.dma_start(
                    out=cadd[:],
                    in_=bass.AP(
                        tensor=st0.tensor,
                        offset=st0.offset,
                        ap=[[1, 128], [128, NSLAB - 1]],
                    ),
                )
                nc.vector.tensor_add(Fk[:, 1:NSLAB], Fk[:, 1:NSLAB], cadd[:])
                nc.gpsimd.dma_start(
                    out=f_out[:, k : k + 1].rearrange("(s p) o -> p (s o)", p=128),
                    in_=Fk[:],
                )

    nc.compile()
    return nc


def _get_program():
    if "nc" not in _CACHE:
        _CACHE["nc"] = _build_program()
    return _CACHE["nc"]


def _prep_in_maps(x, W):
    bf16 = ml_dtypes.bfloat16
    xT = np.ascontiguousarray(x.T).astype(bf16)  # (D, B)
    onehot = (np.arange(128)[:, None] % 64 == np.arange(64)[None, :]).astype(bf16)
    # blockdiag covers only the DVE (relu) planes' m's, per k
    rows = np.arange(KPC * KERNEL_DIM)
    blockdiag = np.zeros((KPC * KERNEL_DIM, KPC), dtype=bf16)
    for k in range(KPC):
        dve_ms = [
            m for m in range(KERNEL_DIM) if (m // 2) not in ACT_PLANES[k]
        ]
        for m in dve_ms:
            blockdiag[k * KERNEL_DIM + m, k] = 1
    selk128 = np.zeros((KPC, KPC * 128), dtype=bf16)
    for k in range(KPC):
        selk128[k, k * 128 : (k + 1) * 128] = 1
    in_maps = []
    for c in range(N_CORES):
        Wc = W[c * KPC : (c + 1) * KPC]  # (KPC, D, M)
        wTc = np.ascontiguousarray(
            Wc.transpose(1, 0, 2).reshape(INPUT_DIM, KPC * KERNEL_DIM)
        )
        in_maps.append(
            {
                "xT": xT,
                "wT": wTc.astype(bf16),
                "onehot": onehot,
                "blockdiag": blockdiag,
                "selk128": selk128,
            }
        )
    return in_maps


def run_hw(x, W, trace=False, **kwargs):
    from concourse.bass_utils import run_bass_kernel_spmd

    nc = _get_program()
    in_maps = _prep_in_maps(x, W)
    res = run_bass_kernel_spmd(
        nc, in_maps, core_ids=list(range(N_CORES)), trace=trace, **kwargs
    )
    F = np.concatenate([res.results[c]["f_out"] for c in range(N_CORES)], axis=1)
    return F.astype(np.float32), res


def kernel(x, W):
    x = np.asarray(x, dtype=np.float32)
    W = np.asarray(W, dtype=np.float32)
    F, _ = run_hw(x, W, trace=False)
    return np.concatenate([x, F], axis=1)


if __name__ == "__main__":
    x = np.random.randn(BATCH, INPUT_DIM).astype(np.float32)
    W = (
        np.random.randn(NB_KERNELS, INPUT_DIM, KERNEL_DIM)
        / np.sqrt(INPUT_DIM + KERNEL_DIM)
    ).astype(np.float32)
    out = kernel(x, W)
    print(out.shape, out.dtype)


# revision 12
# speedup vs baseline: 1.3120x; 1.3120x over previous
"""Trainium2 Bass kernel for MinibatchDiscrimination.

Computes out = concat([x, F], axis=1) where
  act = einsum('bd,kdm->bkm', x, W)               (B, K, M)
  D[b,k,c] = sum_m |act[b,k,m] - act[c,k,m]|      (B, K, B)
  F[b,k] = sum_c exp(-D[b,k,c])                   (B, K)

Sharding: tensor-parallel over nb_kernels (K=32) -> 4 kernels per core on 8
cores. Each core computes its F slice (B, 4); host concatenates [x | F].

Per-core algorithm:
  * actT[(k,m), b] = W_slice^T x^T via PE (bf16 in, f32 accum).
  * Pairwise L1 split between engines: planes on the DVE use
    |x| = 2 relu(x) - x (relu tensor_scalar at 4x) with the -x part
    handled in factored form (rank-1 rs correction + exp bias); planes
    on ACT use Abs directly (|x|/2, needs no correction).  The per-k
    DVE/ACT plane split is asymmetric (k 0,1: ACT gets plane 7; k 2,3:
    planes 6,7) to balance engine load, and blockdiag/rs cover only the
    DVE planes' m values per k.
  * m-contraction on the PE with a one-hot lhsT producing 64 outputs;
    the G=0/G=1 half-slabs go to PSUM column-halves (tile_position
    (0,0)/(0,64)) and are interleaved in issue order so the two column
    groups of the array stream concurrently (~2x PE throughput).
  * The rank-1 rs_c correction opens each slab's accumulation group as a
    single K=4, M=128 matmul.
  * E = exp(-2P + bias) on ACT with accum_out giving row sums.  The
    mirrored column sums accumulate across slabs into one (8, B) PSUM
    tile (lhsT = one-hot column per slab; slab 0 opens rows as zeros),
    copied out 8-wide once per k, bounced via DRAM to transpose into
    per-partition layout, reduced on the DVE.
"""

import sys

if "/opt/trn_rl_repo" not in sys.path:
    sys.path.insert(0, "/opt/trn_rl_repo")

import numpy as np
import ml_dtypes

NB_KERNELS = 32
KERNEL_DIM = 16  # M
INPUT_DIM = 1024  # D
BATCH = 1024  # B
N_CORES = 8
KPC = NB_KERNELS // N_CORES  # kernels per core = 4

# planes handled by ACT (as |x|/2 via Abs) per k; the rest go to the DVE
# (as relu(x)).  blockdiag / the rs correction cover only DVE planes' m's.
ACT_PLANES = {0: (7,), 1: (7,), 2: (6, 7), 3: (6, 7)}

_CACHE = {}


def _build_program():
    import concourse.bass as bass
    import concourse.tile as tile
    from concourse import bacc, mybir

    bf16 = mybir.dt.bfloat16
    f32 = mybir.dt.float32
    Alu = mybir.AluOpType
    Act = mybir.ActivationFunctionType

    nc = bacc.Bacc(
        "TRN2",
        target_bir_lowering=False,
        debug=False,
        enable_asserts=False,
        num_devices=N_CORES,
    )

    KM = KPC * KERNEL_DIM  # 64 rows of actT
    NSLAB = BATCH // 128  # 8 slabs of b

    xT = nc.dram_tensor("xT", (INPUT_DIM, BATCH), bf16, kind="ExternalInput").ap()
    wT = nc.dram_tensor("wT", (INPUT_DIM, KM), bf16, kind="ExternalInput").ap()
    onehot = nc.dram_tensor("onehot", (128, 64), bf16, kind="ExternalInput").ap()
    # blockdiag[(k,m), k'] = 1 if k == k' and m is a DVE plane of k
    blockdiag = nc.dram_tensor("blockdiag", (KM, KPC), bf16, kind="ExternalInput").ap()
    # selk128[q, k*128 + p] = 1 if q == k (rank-1 rs correction, M=128)
    selk128 = nc.dram_tensor("selk128", (KPC, KPC * 128), bf16, kind="ExternalInput").ap()
    f_out = nc.dram_tensor("f_out", (BATCH, KPC), f32, kind="ExternalOutput").ap()

    DCH = INPUT_DIM // 128  # 8 chunks of the matmul contraction dim

    def mm_chunks(span):
        off = 0
        while off < span:
            fd = min(512, span - off)
            yield off, fd
            off += fd

    with tile.TileContext(nc) as tc:
        with (
            tc.tile_pool(name="singles", bufs=1) as singles,
            tc.tile_pool(name="vk", bufs=3) as vk_pool,
            tc.tile_pool(name="sk", bufs=3) as sk_pool,
            tc.tile_pool(name="tg", bufs=8) as t_pool,
            tc.tile_pool(name="es", bufs=3) as e_pool,
            tc.tile_pool(name="fk", bufs=2) as fk_pool,
            tc.tile_pool(name="small", bufs=4) as small_pool,
            tc.tile_pool(name="dps", bufs=2, space="PSUM") as d_psum,
            tc.tile_pool(name="csps", bufs=2, space="PSUM") as cs_psum,
            tc.tile_pool(name="dram", bufs=1, space="DRAM") as dram_pool,
            tc.tile_pool(name="dramk", bufs=2, space="DRAM") as dramk_pool,
        ):
            # ---- Phase 1: actT = wT^T @ xT on PE; rs prep ----
            xT_sb = singles.tile([128, DCH, BATCH], bf16)
            xT_r = xT.rearrange("(i p) b -> p i b", p=128)
            for i in range(DCH):
                eng = nc.scalar if i % 2 else nc.sync
                eng.dma_start(out=xT_sb[:, i, :], in_=xT_r[:, i, :])
            wT_sb = singles.tile([128, DCH, KM], bf16)
            nc.sync.dma_start(out=wT_sb[:], in_=wT.rearrange("(i p) c -> p i c", p=128))
            onehot_sb = singles.tile([128, 64], bf16)
            nc.sync.dma_start(out=onehot_sb[:], in_=onehot)
            blockdiag_sb = singles.tile([KM, KPC], bf16)
            nc.sync.dma_start(out=blockdiag_sb[:], in_=blockdiag)
            selk128_sb = singles.tile([KPC, KPC * 128], bf16)
            nc.sync.dma_start(out=selk128_sb[:], in_=selk128)
            ones_sb = singles.tile([128, 1], bf16)
            nc.vector.memset(ones_sb[:], 1.0)

            actT_sb = singles.tile([KM, BATCH], bf16)
            rsh_bf = singles.tile([KPC, BATCH], bf16)  # bf16(-rs/2)
            rs2_bf = singles.tile([KPC, BATCH], bf16)  # bf16(-rs) == 2*rsh_bf
            act_ps = d_psum.tile([KM, BATCH], f32, tag="D")
            for h in range(BATCH // 512):
                for i in range(DCH):
                    nc.tensor.matmul(
                        act_ps[:, h * 512 : (h + 1) * 512],
                        lhsT=wT_sb[:, i, :],
                        rhs=xT_sb[:, i, h * 512 : (h + 1) * 512],
                        start=(i == 0),
                        stop=(i == DCH - 1),
                    )
            nc.scalar.copy(actT_sb[:], act_ps[:])
            # rs[k, c] = sum over k's DVE-plane m's of bf16 actT[c, k, m]
            rs_ps = d_psum.tile([KPC, BATCH], f32, tag="D")
            for h in range(BATCH // 512):
                nc.tensor.matmul(
                    rs_ps[:, h * 512 : (h + 1) * 512],
                    lhsT=blockdiag_sb[:],
                    rhs=actT_sb[:, h * 512 : (h + 1) * 512],
                    start=True,
                    stop=True,
                )
            # bf16 rounding commutes with *2, so 2*rsh_bf == rs2_bf exactly;
            # the exp argument on the diagonal cancels to 0.
            nc.scalar.mul(rsh_bf[:], rs_ps[:], -0.5)
            nc.scalar.mul(rs2_bf[:], rs_ps[:], -1.0)

            # f32 upconvert of the bf16-rounded actT (tensor_scalar scalars
            # must be f32 but must equal V's bf16 values exactly).
            actT_f32 = singles.tile([KM, BATCH], f32)
            nc.vector.tensor_copy(actT_f32[:], actT_sb[:])

            # DRAM bounces: the broadcast/gather DMAs below need arbitrary
            # strided (incl. 0-step) source APs, which SBUF sources disallow.
            actT_dram = dram_pool.tile([KM, BATCH], bf16)
            nc.sync.dma_start(out=actT_dram[:], in_=actT_sb[:])
            actT32_dram = dram_pool.tile([KM, BATCH], f32)
            nc.sync.dma_start(out=actT32_dram[:], in_=actT_f32[:])
            rs2_dram = dram_pool.tile([KPC, BATCH], bf16)
            nc.sync.dma_start(out=rs2_dram[:], in_=rs2_bf[:])

            # bias_cols[p, k*NSLAB + slab] = bf16(-rs[k, slab*128 + p])
            bias_cols = singles.tile([128, KPC * NSLAB], bf16)
            rc0 = rs2_dram[0:1, 0:1]
            nc.sync.dma_start(
                out=bias_cols[:],
                in_=bass.AP(
                    tensor=rc0.tensor,
                    offset=rc0.offset,
                    ap=[[1, 128], [BATCH, KPC], [128, NSLAB]],
                ),
            )

            # ---- Phases 2+3 per kernel k ----
            # Partition layout of T tiles, per plane j in 0..7:
            #   partition p holds m = 2j + p//64 and b_loc = p % 64.
            # V[p, j, c] = actT_bf16[k*16 + 2j + p//64, c]
            # S[p, j, f] = actT_f32 [k*16 + 2j + p//64,
            #                        slab*128 + G*64 + p%64],  f = slab*2+G
            for k in range(KPC):
                base = k * KERNEL_DIM
                act_js = ACT_PLANES[k]
                # ACT planes first so the slower producer doesn't gate the
                # accumulation-group closers.
                j_order = list(act_js) + [j for j in range(8) if j not in act_js]
                Vp = {}
                S = sk_pool.tile([128, 8, 16], f32)
                # trigger cost on the issuing queue scales with the number of
                # destination partition-lines (~12-30ns each), so spread the
                # V broadcasts (64 lines each) and S gathers round-robin over
                # all three DMA-capable queues.
                dma_engs = (nc.sync, nc.scalar, nc.gpsimd)
                rr = 0
                for j in j_order:
                    Vp[j] = vk_pool.tile(
                        [128, BATCH], bf16, name=f"V{k}_{j}", tag=f"v{j}"
                    )
                    for q in range(2):
                        row = base + 2 * j + q
                        row_b = actT_dram[row : row + 1, 0:1]
                        dma_engs[rr % 3].dma_start(
                            out=Vp[j][64 * q : 64 * (q + 1), :],
                            in_=bass.AP(
                                tensor=row_b.tensor,
                                offset=row_b.offset,
                                ap=[[0, 64], [1, BATCH]],
                            ),
                        )
                        rr += 1
                        row_s = actT32_dram[row : row + 1, 0:1]
                        dma_engs[rr % 3].dma_start(
                            out=S[64 * q : 64 * (q + 1), j, :],
                            in_=bass.AP(
                                tensor=row_s.tensor,
                                offset=row_s.offset,
                                ap=[[1, 64], [128, NSLAB], [64, 2]],
                            ),
                        )
                        rr += 1
                # S_nh = -S/2: ACT-plane bias (Abs(0.5*V - 0.5*s) = |x|/2)
                S_nh = sk_pool.tile([128, 8, 16], f32, tag="snh")
                nc.vector.tensor_scalar(
                    out=S_nh.rearrange("p a b -> p (a b)"),
                    in0=S.rearrange("p a b -> p (a b)"),
                    scalar1=-0.5,
                    scalar2=None,
                    op0=Alu.mult,
                )

                # column-sum accumulator: cs_acc[0, c] accumulates, across
                # slabs s with 128*(s+1) <= c, the mirror mass
                # sum_b E_s[b, c].  One accumulation group spans the whole k
                # loop (slab 0 opens all of [128, B), slab 6 closes).
                cs_acc = cs_psum.tile([1, BATCH], f32)
                Fk = fk_pool.tile([128, NSLAB], f32)

                for slab in range(NSLAB):
                    c0 = slab * 128
                    span = BATCH - c0
                    # PSUM accumulates P = R + bf16(-rs[c]/2) (DVE planes)
                    # + sum |x|/2 (ACT planes).  Then exp(-2P + bias[b]) with
                    # bias[b] = bf16(-rs[b]) gives exp(-sum_m |act_b-act_c|),
                    # exactly 1 on the diagonal.
                    D = d_psum.tile([128, BATCH], f32)
                    # rank-1 rs correction opens the group across all 128
                    # partitions in one K=4, M=128 matmul.
                    for off, fd in mm_chunks(span):
                        nc.tensor.matmul(
                            D[:, off : off + fd],
                            lhsT=selk128_sb[:, k * 128 : (k + 1) * 128],
                            rhs=rsh_bf[:, c0 + off : c0 + off + fd],
                            start=True,
                            stop=False,
                        )
                    for j in j_order:
                        Tg = {}
                        for G in range(2):
                            scol = slab * 2 + G
                            T = t_pool.tile([128, BATCH], bf16, tag=f"t{G}")
                            if j in act_js:
                                nc.scalar.activation(
                                    out=T[:, :span],
                                    in_=Vp[j][:, c0:BATCH],
                                    func=Act.Abs,
                                    scale=0.5,
                                    bias=S_nh[:, j, scol : scol + 1],
                                )
                            else:
                                nc.vector.tensor_scalar(
                                    out=T[:, :span],
                                    in0=Vp[j][:, c0:BATCH],
                                    scalar1=S[:, j, scol : scol + 1],
                                    scalar2=0.0,
                                    op0=Alu.subtract,
                                    op1=Alu.max,
                                )
                            Tg[G] = T
                        last = j == j_order[-1]
                        # adjacent matmuls alternate PSUM column-halves
                        # (tile_position (0,0)/(0,64)) -> both column groups
                        # of the PE array stream concurrently.
                        for off, fd in mm_chunks(span):
                            for G in range(2):
                                nc.tensor.matmul(
                                    D[G * 64 : (G + 1) * 64, off : off + fd],
                                    lhsT=onehot_sb[:],
                                    rhs=Tg[G][:, off : off + fd],
                                    start=False,
                                    stop=last,
                                )
                    # E = exp(-D) over the slab's c-window; accum_out gives
                    # the row part sum_c E directly.
                    E = e_pool.tile([128, BATCH], bf16)
                    nc.scalar.activation(
                        out=E[:, :span],
                        in_=D[:, :span],
                        func=Act.Exp,
                        scale=-2.0,
                        bias=bias_cols[:, k * NSLAB + slab : k * NSLAB + slab + 1],
                        accum_out=Fk[:, slab : slab + 1],
                    )
                    # mirrored column parts for c in [c0+128, B): accumulate
                    # into the cross-slab cs_acc row.
                    if span > 128:
                        start = c0 + 128
                        while start < BATCH:
                            # keep each matmul within one PSUM bank (512 f32)
                            fd = min(512 - (start % 512), BATCH - start)
                            nc.tensor.matmul(
                                cs_acc[:, start : start + fd],
                                lhsT=ones_sb[:],
                                rhs=E[:, start - c0 : start - c0 + fd],
                                start=(slab == 0),
                                stop=(slab == NSLAB - 2)
                                and (start + fd >= BATCH),
                            )
                            start += fd

                # bounce the accumulated mirror row via DRAM into the
                # per-partition (p, t) layout and add into Fk
                colrow = small_pool.tile([1, BATCH - 128], f32, tag="colrow")
                nc.scalar.copy(colrow[:], cs_acc[0:1, 128:BATCH])
                rowdram = dramk_pool.tile([1, BATCH - 128], f32, tag="stg")
                nc.scalar.dma_start(out=rowdram[:], in_=colrow[:])
                # cadd[p, t] = rowdram[128*t + p] (t = 1..7 -> idx 0..6)
                cadd = small_pool.tile([128, NSLAB - 1], f32)
                st0 = rowdram[0:1, 0:1]
                nc.scalar.dma_start(
                    out=cadd[:],
                    in_=bass.AP(
                        tensor=st0.tensor,
                        offset=st0.offset,
                        ap=[[1, 128], [128, NSLAB - 1]],
                    ),
                )
                nc.vector.tensor_add(Fk[:, 1:NSLAB], Fk[:, 1:NSLAB], cadd[:])
                nc.gpsimd.dma_start(
                    out=f_out[:, k : k + 1].rearrange("(s p) o -> p (s o)", p=128),
                    in_=Fk[:],
                )

    nc.compile()
    return nc


def _get_program():
    if "nc" not in _CACHE:
        _CACHE["nc"] = _build_program()
    return _CACHE["nc"]


def _prep_in_maps(x, W):
    bf16 = ml_dtypes.bfloat16
    xT = np.ascontiguousarray(x.T).astype(bf16)  # (D, B)
    onehot = (np.arange(128)[:, None] % 64 == np.arange(64)[None, :]).astype(bf16)
    # blockdiag covers only the DVE (relu) planes' m's, per k
    rows = np.arange(KPC * KERNEL_DIM)
    blockdiag = np.zeros((KPC * KERNEL_DIM, KPC), dtype=bf16)
    for k in range(KPC):
        dve_ms = [
            m for m in range(KERNEL_DIM) if (m // 2) not in ACT_PLANES[k]
        ]
        for m in dve_ms:
            blockdiag[k * KERNEL_DIM + m, k] = 1
    selk128 = np.zeros((KPC, KPC * 128), dtype=bf16)
    for k in range(KPC):
        selk128[k, k * 128 : (k + 1) * 128] = 1
    in_maps = []
    for c in range(N_CORES):
        Wc = W[c * KPC : (c + 1) * KPC]  # (KPC, D, M)
        wTc = np.ascontiguousarray(
            Wc.transpose(1, 0, 2).reshape(INPUT_DIM, KPC * KERNEL_DIM)
        )
        in_maps.append(
            {
                "xT": xT,
                "wT": wTc.astype(bf16),
                "onehot": onehot,
                "blockdiag": blockdiag,
                "selk128": selk128,
            }
        )
    return in_maps


def run_hw(x, W, trace=False, **kwargs):
    from concourse.bass_utils import run_bass_kernel_spmd

    nc = _get_program()
    in_maps = _prep_in_maps(x, W)
    res = run_bass_kernel_spmd(
        nc, in_maps, core_ids=list(range(N_CORES)), trace=trace, **kwargs
    )
    F = np.concatenate([res.results[c]["f_out"] for c in range(N_CORES)], axis=1)
    return F.astype(np.float32), res


def kernel(x, W):
    x = np.asarray(x, dtype=np.float32)
    W = np.asarray(W, dtype=np.float32)
    F, _ = run_hw(x, W, trace=False)
    return np.concatenate([x, F], axis=1)


if __name__ == "__main__":
    x = np.random.randn(BATCH, INPUT_DIM).astype(np.float32)
    W = (
        np.random.randn(NB_KERNELS, INPUT_DIM, KERNEL_DIM)
        / np.sqrt(INPUT_DIM + KERNEL_DIM)
    ).astype(np.float32)
    out = kernel(x, W)
    print(out.shape, out.dtype)
